# revision 13
# baseline (speedup 1.0000x reference)
"""Trainium2 Bass kernel for nn_EquivariantMultiHeadAttention.

Strategy (8 cores, fully independent — no collectives):
  - Nodes are grouped into windows of 128. Windows are rank-matched by edge
    count and dealt to (core, slot) pairs so every core runs the same static
    program: SLOTS slots x B_s blocks of 128 edges.
  - Each core computes k,v for ALL nodes (replicated compute) into a DRAM
    [N, 512] record, then per edge block gathers k|v rows via dma_gather,
    computes attention messages, and scatter-adds them into a per-window
    PSUM accumulator with a one-hot matmul. dst-side q is selected from the
    window's q tile with the transposed one-hot (PE matmul, no DRAM gather).
  - vec[src] rows are host-gathered into a per-core edge-aligned stream
    (sequential DMA on device instead of a random gather).
  - Window finalize: output projection + equivariant combine, write dx/dvec.
"""

import sys
import numpy as np

sys.path.insert(0, "/opt/trn_rl_repo")

import concourse.bass as bass
import concourse.bacc as bacc
import concourse.mybir as mybir
import concourse.tile as tile
from concourse.masks import make_identity

N, E, HC, NH, HD, RBF = 32768, 262144, 128, 8, 16, 64
CUT_HI, EPS = 5.0, 1e-5
CORES, P = 8, 128
SLOTS = 32
CHUNK = 4  # blocks per kv-gather / vec load

F32 = mybir.dt.float32
BF16 = mybir.dt.bfloat16
F32R = mybir.dt.float32r
I16 = mybir.dt.int16

# data dtype mode: "f32" (exact), "f32r" (f32 tiles, fast matmul), "bf16"
DT_MODE = "f32"
PROFILE = False
TRACE_KW = {}


def _np_dt(dt):
    return np.dtype(mybir.dt.np(dt))


class Cfg:
    pass


def _prep(inputs, dt_mode):
    """Host-side sharding/staging. Returns (cfg, in_maps, meta)."""
    inp = {k: np.asarray(v) for k, v in inputs.items()}
    x = inp["x"].astype(np.float32)
    vec = inp["vec"].astype(np.float32)
    src = inp["edge_index"][0].astype(np.int64)
    dst = inp["edge_index"][1].astype(np.int64)
    r_ij = inp["r_ij"].astype(np.float32)
    f_ij = inp["f_ij"].astype(np.float32)
    d_ij = inp["d_ij"].astype(np.float32)
    n_all = x.shape[0]
    n_win = n_all // P
    assert n_win == CORES * SLOTS

    dt = BF16 if dt_mode == "bf16" else F32
    npdt = _np_dt(dt)

    # ---- fold LayerNorm affine into projection weights ----
    g, b = inp["ln_g"].astype(np.float64), inp["ln_b"].astype(np.float64)
    Wq = inp["Wq"].astype(np.float64)
    Wk = inp["Wk"].astype(np.float64)
    Wv = inp["Wv"].astype(np.float64)
    Wo = inp["Wo"].astype(np.float64)
    Wvec = inp["Wvec"].astype(np.float64)
    Wdk = inp["Wdk"].astype(np.float64)
    Wdv = inp["Wdv"].astype(np.float64)
    wqT = (Wq * g[None, :]).T  # [HC, HC]
    bq = inp["bq"].astype(np.float64) + Wq @ b
    wkT = (Wk * g[None, :]).T
    bk = inp["bk"].astype(np.float64) + Wk @ b
    wvT = (Wv * g[None, :]).T  # [HC, 3HC]
    bv = inp["bv"].astype(np.float64) + Wv @ b
    wkvT = np.concatenate([wkT, wvT], axis=1)  # [HC, 512]
    bias_kv = np.concatenate([bk, bv])  # [512]
    wdkvT = np.concatenate([Wdk.T, Wdv.T], axis=1)  # [RBF, 512]
    bias_dkv = np.concatenate([inp["bdk"], inp["bdv"]]).astype(np.float64)
    woT = Wo.T  # [HC, 3HC]
    bo = inp["bo"].astype(np.float64)
    wvecT = Wvec.T  # [HC, 3HC]

    # ---- window -> (core, slot) assignment (rank-matched for balance) ----
    win = dst >> 7
    counts = np.bincount(win, minlength=n_win)
    order = np.argsort(counts, kind="stable")
    win_of = np.zeros((CORES, SLOTS), dtype=np.int64)
    B_s = np.zeros(SLOTS, dtype=np.int64)
    for s in range(SLOTS):
        grp = order[s * CORES:(s + 1) * CORES]
        B_s[s] = max(1, int(np.ceil(counts[grp].max() / P)))
        cs = range(CORES) if s % 2 == 0 else range(CORES - 1, -1, -1)
        for i, c in enumerate(cs):
            win_of[c, s] = grp[i]
    NB = int(B_s.sum())
    E_pad = NB * P

    eorder = np.argsort(win, kind="stable")
    wstart = np.zeros(n_win + 1, dtype=np.int64)
    np.cumsum(np.bincount(win, minlength=n_win), out=wstart[1:])

    cfg = Cfg()
    cfg.B_s = [int(v) for v in B_s]
    cfg.NB = NB
    cfg.E_pad = E_pad
    cfg.n_all = n_all
    cfg.npc = SLOTS * P
    cfg.slots = SLOTS
    cfg.n_tiles = n_all // P
    cfg.dt_mode = dt_mode
    cfg.has_dkv_bias = bool(np.any(bias_dkv != 0.0))

    in_maps = []
    meta = {"node_ids": []}
    for c in range(CORES):
        src_pad = np.zeros(E_pad, dtype=np.int16)
        dstw = np.full(E_pad, -1.0, dtype=np.float32)
        r_pad = np.full(E_pad, CUT_HI, dtype=np.float32)
        d_pad = np.zeros((E_pad, 3), dtype=np.float32)
        f_pad = np.zeros((E_pad, RBF), dtype=np.float32)
        vg = np.zeros((E_pad, 3 * HC), dtype=np.float32)
        node_ids = np.zeros(cfg.npc, dtype=np.int64)
        off = 0
        for s in range(SLOTS):
            gwin = win_of[c, s]
            eids = eorder[wstart[gwin]:wstart[gwin + 1]]
            ne = len(eids)
            assert ne <= B_s[s] * P
            sl = slice(off, off + ne)
            src_pad[sl] = src[eids].astype(np.int16)
            dstw[sl] = (dst[eids] - gwin * P).astype(np.float32)
            r_pad[sl] = r_ij[eids]
            d_pad[sl] = d_ij[eids]
            f_pad[sl] = f_ij[eids]
            vg[sl] = vec[src[eids]].reshape(ne, 3 * HC)
            node_ids[s * P:(s + 1) * P] = np.arange(gwin * P, (gwin + 1) * P)
            off += B_s[s] * P
        assert off == E_pad
        meta["node_ids"].append(node_ids)

        emisc = np.stack([dstw, r_pad, d_pad[:, 0], d_pad[:, 1], d_pad[:, 2]],
                         axis=1)  # [E_pad, 5]
        emisc = emisc.reshape(NB, P, 5).transpose(1, 0, 2).reshape(P, NB * 5)
        idx16 = np.tile(src_pad.reshape(-1, 16).T, (8, 1))  # [128, E_pad/16]
        fT = np.ascontiguousarray(f_pad.T).astype(npdt)  # [RBF, E_pad]

        m = {
            "x": x,
            "x_own": np.ascontiguousarray(x[node_ids]),
            "vec_own": np.ascontiguousarray(vec[node_ids]),
            "vec_g": vg.astype(npdt),
            "fT": fT,
            "emisc": np.ascontiguousarray(emisc),
            "idx16": np.ascontiguousarray(idx16),
            "wkvT": wkvT.astype(npdt),
            "wqT": wqT.astype(npdt),
            "wdkvT": wdkvT.astype(npdt),
            "wvecT": wvecT.astype(npdt),
            "woT": woT.astype(npdt),
            "bias_kv": np.tile(bias_kv.astype(np.float32)[None, :], (P, 1)),
            "bias_q": np.tile(bq.astype(np.float32)[None, :], (P, 1)),
            "bias_o": np.tile(bo.astype(np.float32)[None, :], (P, 1)),
        }
        if cfg.has_dkv_bias:
            m["bias_dkv"] = np.ascontiguousarray(
                bias_dkv.astype(npdt)[None, :])
        in_maps.append(m)
    return cfg, in_maps, meta


def build(cfg):
    dt_mode = cfg.dt_mode
    dt = BF16 if dt_mode == "bf16" else F32
    mdt = {"bf16": BF16, "f32r": F32R, "f32": F32}[dt_mode]

    def mmcast(ap):
        return ap

    nc = bacc.Bacc("TRN2", target_bir_lowering=False, debug=False)

    n_all, NB, E_pad = cfg.n_all, cfg.NB, cfg.E_pad
    HC3 = 3 * HC

    x_d = nc.dram_tensor("x", [n_all, HC], F32, kind="ExternalInput")
    xo_d = nc.dram_tensor("x_own", [cfg.npc, HC], F32, kind="ExternalInput")
    vo_d = nc.dram_tensor("vec_own", [cfg.npc, 3, HC], F32, kind="ExternalInput")
    vg_d = nc.dram_tensor("vec_g", [E_pad, HC3], dt, kind="ExternalInput")
    fT_d = nc.dram_tensor("fT", [RBF, E_pad], mdt, kind="ExternalInput")
    em_d = nc.dram_tensor("emisc", [P, NB * 5], F32, kind="ExternalInput")
    ix_d = nc.dram_tensor("idx16", [P, E_pad // 16], I16, kind="ExternalInput")
    wkvT_d = nc.dram_tensor("wkvT", [HC, 512], mdt, kind="ExternalInput")
    wqT_d = nc.dram_tensor("wqT", [HC, HC], mdt, kind="ExternalInput")
    wdkvT_d = nc.dram_tensor("wdkvT", [RBF, 512], mdt, kind="ExternalInput")
    wvecT_d = nc.dram_tensor("wvecT", [HC, HC3], mdt, kind="ExternalInput")
    woT_d = nc.dram_tensor("woT", [HC, HC3], mdt, kind="ExternalInput")
    bkv_d = nc.dram_tensor("bias_kv", [P, 512], F32, kind="ExternalInput")
    bq_d = nc.dram_tensor("bias_q", [P, HC], F32, kind="ExternalInput")
    bo_d = nc.dram_tensor("bias_o", [P, HC3], F32, kind="ExternalInput")
    if cfg.has_dkv_bias:
        bdkv_d = nc.dram_tensor("bias_dkv", [1, 512], mdt, kind="ExternalInput")

    kv_d = nc.dram_tensor("kv_tab", [n_all, 512], dt, kind="Internal")
    dx_d = nc.dram_tensor("dx", [cfg.npc, HC], F32, kind="ExternalOutput")
    dvec_d = nc.dram_tensor("dvec", [cfg.npc, 3, HC], F32, kind="ExternalOutput")

    TT = mybir.AluOpType
    AF = mybir.ActivationFunctionType
    qwins = []

    with tile.TileContext(nc) as tc:
        cpool = tc.alloc_tile_pool(name="const", bufs=1)
        qpool = tc.alloc_tile_pool(name="qwin", bufs=cfg.slots)
        npool = tc.alloc_tile_pool(name="node", bufs=3)
        epool = tc.alloc_tile_pool(name="edge", bufs=3)
        gpool = tc.alloc_tile_pool(name="gath", bufs=2)
        fpool = tc.alloc_tile_pool(name="fin", bufs=2)

        # ---------- constants ----------
        iota_f32 = cpool.tile([P, P], F32, tag="iota")
        nc.gpsimd.iota(iota_f32[:], pattern=[[1, P]], base=0,
                       channel_multiplier=0,
                       allow_small_or_imprecise_dtypes=True)
        eps_t = cpool.tile([P, 1], F32, tag="eps_t")
        nc.vector.memset(eps_t[:], EPS)
        halfpi_t = cpool.tile([P, 1], F32, tag="halfpi_t")
        nc.vector.memset(halfpi_t[:], float(np.pi / 2))
        if mdt != F32:
            ident_f = cpool.tile([P, P], F32, tag="identf")
            make_identity(nc, ident_f[:])
            ident = cpool.tile([P, P], mdt, tag="ident")
            nc.scalar.copy(ident[:], ident_f[:])
        else:
            ident_f = cpool.tile([P, P], F32, tag="identf")
            make_identity(nc, ident_f[:])
            ident = ident_f

        def cload(name, shape, dtt, dram):
            t = cpool.tile(shape, dtt, tag=name)
            nc.sync.dma_start(t[:], dram[:])
            return t

        wkvT = cload("wkvT", [HC, 512], mdt, wkvT_d)
        wqT = cload("wqT", [HC, HC], mdt, wqT_d)
        wdkvT = cload("wdkvT", [RBF, 512], mdt, wdkvT_d)
        wvecT = cload("wvecT", [HC, HC3], mdt, wvecT_d)
        woT = cload("woT", [HC, HC3], mdt, woT_d)
        bias_kv = cload("bias_kv", [P, 512], F32, bkv_d)
        bias_q = cload("bias_q", [P, HC], F32, bq_d)
        bias_o = cload("bias_o", [P, HC3], F32, bo_d)
        emisc = cload("emisc", [P, NB * 5], F32, em_d)
        idx16 = cload("idx16", [P, E_pad // 16], I16, ix_d)
        if cfg.has_dkv_bias:
            bias_dkv = cload("bias_dkv", [1, 512], mdt, bdkv_d)
            ones1f = cpool.tile([1, P], F32, tag="ones1f")
            nc.vector.memset(ones1f[:], 1.0)
            ones1 = cpool.tile([1, P], mdt, tag="ones1")
            nc.scalar.copy(ones1[:], ones1f[:])

        # ---------- LayerNorm helper ----------
        def layer_norm(pool, xt):
            """xt: [P, HC] f32 SBUF -> xn [P, HC] dt SBUF"""
            st = pool.tile([P, 6], F32, tag="ln_st")
            ag = pool.tile([P, 2], F32, tag="ln_ag")
            nc.vector.bn_stats(st[:], xt[:])
            nc.vector.bn_aggr(ag[:], st[:])
            std = pool.tile([P, 1], F32, tag="ln_std")
            nc.scalar.activation(std[:], ag[:, 1:2], AF.Sqrt, bias=eps_t[:])
            rstd = pool.tile([P, 1], F32, tag="ln_rstd")
            nc.vector.reciprocal(rstd[:], std[:])
            mb = pool.tile([P, 1], F32, tag="ln_mb")
            nc.vector.tensor_scalar(mb[:], ag[:, 0:1], rstd[:], -1.0,
                                    op0=TT.mult, op1=TT.mult)
            xn = pool.tile([P, HC], mdt, tag="ln_xn")
            nc.scalar.activation(xn[:], xt[:], AF.Identity,
                                 bias=mb[:], scale=rstd[:])
            return xn

        def transpose_to(pool, psum_pool, src_ap, dtt, idt, tag):
            ps = psum_pool.tile([P, P], src_ap.dtype, tag="trps", space="PSUM")
            nc.tensor.transpose(ps[:], src_ap, idt[:])
            out = pool.tile([P, P], dtt, tag=tag)
            nc.scalar.copy(out[:], ps[:])
            return out

        # ---------- node phase: k,v for all nodes; q for own windows ----------
        with tc.tile_pool(name="nps", bufs=2, space="PSUM") as nps:
            for t in range(cfg.n_tiles):
                xt = npool.tile([P, HC], F32, tag="xt")
                nc.sync.dma_start(xt[:], x_d[t * P:(t + 1) * P, :])
                xn = layer_norm(npool, xt)
                xT = transpose_to(npool, nps, xn[:], mdt, ident, "xT")
                ps = nps.tile([P, 512], F32, tag="mmps", space="PSUM")
                nc.tensor.matmul(ps[:], mmcast(xT[:]), mmcast(wkvT[:]),
                                 start=True, stop=True)
                kv = npool.tile([P, 512], dt, tag="kvsb")
                nc.vector.tensor_tensor(kv[:], ps[:], bias_kv[:], op=TT.add)
                nc.sync.dma_start(kv_d[t * P:(t + 1) * P, :], kv[:])

            for s in range(cfg.slots):
                xt = npool.tile([P, HC], F32, tag="xt")
                nc.sync.dma_start(xt[:], xo_d[s * P:(s + 1) * P, :])
                xn = layer_norm(npool, xt)
                xT = transpose_to(npool, nps, xn[:], mdt, ident, "xT")
                ps = nps.tile([P, 512], F32, tag="mmps", space="PSUM")
                nc.tensor.matmul(ps[:, :HC], mmcast(xT[:]), mmcast(wqT[:]),
                                 start=True, stop=True)
                qw = qpool.tile([P, HC], mdt, tag="qwin")
                nc.vector.tensor_tensor(qw[:], ps[:, :HC], bias_q[:],
                                        op=TT.add)
                qwins.append(qw)

        # ---------- edge phase ----------
        with tc.tile_pool(name="eps", bufs=1, space="PSUM") as eps, \
             tc.tile_pool(name="eps2", bufs=2, space="PSUM") as eps2:
            off = 0
            bb = 0
            for s in range(cfg.slots):
                B = cfg.B_s[s]
                fTs = gpool.tile([RBF, B * P], mdt, tag="fTs")
                nc.sync.dma_start(fTs[:], fT_d[:, off:off + B * P])
                agg = eps.tile([P, 512], F32, tag="agg", space="PSUM")
                qw = qwins[s]

                jb = 0
                while jb < B:
                    ch = min(CHUNK, B - jb)
                    G = ch * P
                    o = off + jb * P
                    kvg = gpool.tile([P, ch, 512], dt, tag="kvg")
                    nc.gpsimd.dma_gather(
                        kvg[:], kv_d[:, :], idx16[:, o // 16:(o + G) // 16],
                        G, G, 512)
                    vg = gpool.tile([P, ch, HC3], dt, tag="vg")
                    nc.sync.dma_start(
                        vg[:],
                        vg_d[o:o + G, :].rearrange("(c p) f -> p c f", p=P))

                    for j in range(ch):
                        blk = bb + jb + j
                        dcol = emisc[:, blk * 5 + 2:blk * 5 + 5]
                        dstw = emisc[:, blk * 5:blk * 5 + 1]
                        rr = emisc[:, blk * 5 + 1:blk * 5 + 2]

                        onehot = epool.tile([P, P], mdt, tag="onehot")
                        nc.vector.tensor_scalar(onehot[:], iota_f32[:],
                                                dstw, None, op0=TT.is_equal)
                        ohT = transpose_to(epool, eps2, onehot[:], mdt, ident,
                                           "ohT")
                        qsel = eps2.tile([P, HC], F32, tag="qsel", space="PSUM")
                        nc.tensor.matmul(qsel[:], mmcast(ohT[:]),
                                         mmcast(qw[:]), start=True, stop=True)

                        dkv = eps2.tile([P, 512], F32, tag="dkv", space="PSUM")
                        fcol = fTs[:, (jb + j) * P:(jb + j + 1) * P]
                        if cfg.has_dkv_bias:
                            nc.tensor.matmul(dkv[:], mmcast(ones1[:]),
                                             mmcast(bias_dkv[:]), start=True,
                                             stop=False)
                            nc.tensor.matmul(dkv[:], mmcast(fcol),
                                             mmcast(wdkvT[:]), start=False,
                                             stop=True)
                        else:
                            nc.tensor.matmul(dkv[:], mmcast(fcol),
                                             mmcast(wdkvT[:]), start=True,
                                             stop=True)
                        dk = epool.tile([P, HC], dt, tag="dk")
                        nc.scalar.activation(dk[:], dkv[:, :HC], AF.Silu)
                        dv = epool.tile([P, 384], dt, tag="dv")
                        nc.scalar.activation(dv[:], dkv[:, HC:512], AF.Silu)

                        kk = epool.tile([P, HC], dt, tag="kk")
                        nc.vector.tensor_tensor(kk[:], kvg[:, j, 0:HC], dk[:],
                                                op=TT.mult)
                        qkk = epool.tile([P, HC], dt, tag="qkk")
                        nc.vector.tensor_tensor(qkk[:], qsel[:], kk[:],
                                                op=TT.mult)
                        attn = epool.tile([P, NH], F32, tag="attn")
                        nc.vector.tensor_reduce(
                            attn[:],
                            qkk[:].rearrange("p (h d) -> p h d", d=HD),
                            axis=mybir.AxisListType.X, op=TT.add)
                        # cutoff: 0.5*cos(r*pi/5)+0.5 = 0.5*sin(r*pi/5+pi/2)+0.5
                        cut = epool.tile([P, 1], F32, tag="cut")
                        # cos(t) = sin(pi/2 - t), arg stays in [-pi/2, pi/2]
                        nc.scalar.activation(cut[:], rr, AF.Sin,
                                             bias=halfpi_t[:],
                                             scale=float(-np.pi / CUT_HI))
                        cutp = epool.tile([P, 1], F32, tag="cutp")
                        nc.scalar.activation(cutp[:], cut[:], AF.Copy,
                                             bias=0.5, scale=0.5)
                        attn_s = epool.tile([P, NH], F32, tag="attn_s")
                        nc.scalar.activation(attn_s[:], attn[:], AF.Silu)
                        attn2 = epool.tile([P, NH], F32, tag="attn2")
                        nc.vector.tensor_scalar(attn2[:], attn_s[:], cutp[:],
                                                None, op0=TT.mult)

                        vjdv = epool.tile([P, 384], dt, tag="vjdv")
                        nc.vector.tensor_tensor(vjdv[:], kvg[:, j, HC:512],
                                                dv[:], op=TT.mult)
                        v3 = vjdv[:].rearrange("p (h t) -> p h t", t=3 * HD)
                        msg = epool.tile([P, 512], mdt, tag="msg")
                        nc.vector.tensor_tensor(
                            msg[:, 0:HC].rearrange("p (h d) -> p h d", d=HD),
                            v3[:, :, 0:HD],
                            attn2[:].rearrange("p (h o) -> p h o",
                                               o=1).to_broadcast([P, NH, HD]),
                            op=TT.mult)
                        for c3 in range(3):
                            tmp = epool.tile([P, HC], dt, tag=f"tmp{c3}")
                            nc.scalar.activation(
                                tmp[:].rearrange("p (h d) -> p h d", d=HD),
                                v3[:, :, 2 * HD:3 * HD], AF.Copy,
                                scale=dcol[:, c3:c3 + 1])
                            mslc = msg[:, HC + c3 * HC:HC + (c3 + 1) * HC]
                            nc.vector.tensor_tensor(
                                mslc.rearrange("p (h d) -> p h d", d=HD),
                                vg[:, j, c3 * HC:(c3 + 1) * HC].rearrange(
                                    "p (h d) -> p h d", d=HD),
                                v3[:, :, HD:2 * HD], op=TT.mult)
                            nc.vector.tensor_tensor(mslc, mslc, tmp[:],
                                                    op=TT.add)
                        nc.tensor.matmul(agg[:], mmcast(onehot[:]),
                                         mmcast(msg[:]),
                                         start=(jb + j == 0),
                                         stop=(jb + j == B - 1))
                    jb += ch

                # ---------- finalize slot s ----------
                aggs = fpool.tile([P, 512], F32, tag="aggs")
                nc.vector.tensor_copy(aggs[:], agg[:])
                xaT = transpose_to(fpool, eps2, aggs[:, 0:HC], mdt, ident_f,
                                   "xaT")
                ops = eps2.tile([P, HC3], F32, tag="qsel", space="PSUM")
                nc.tensor.matmul(ops[:], mmcast(xaT[:]), mmcast(woT[:]),
                                 start=True, stop=True)
                o_sb = fpool.tile([P, HC3], F32, tag="o_sb")
                nc.vector.tensor_tensor(o_sb[:], ops[:], bias_o[:], op=TT.add)

                # vp = vec_own @ WvecT per component
                vot = fpool.tile([P, 3 * HC], F32, tag="vot")
                nc.sync.dma_start(
                    vot[:],
                    vo_d[s * P:(s + 1) * P, :, :].rearrange("n c f -> n (c f)"))
                vd = fpool.tile([P, HC], F32, tag="vd")
                vec3 = fpool.tile([P, HC3], F32, tag="vec3")
                for c3 in range(3):
                    vT = transpose_to(fpool, eps2,
                                      vot[:, c3 * HC:(c3 + 1) * HC], mdt,
                                      ident_f, "vT")
                    vpps = eps2.tile([P, HC3], F32, tag="dkv", space="PSUM")
                    nc.tensor.matmul(vpps[:], mmcast(vT[:]), mmcast(wvecT[:]),
                                     start=True, stop=True)
                    vp_sb = fpool.tile([P, HC3], F32, tag="vp_sb")
                    nc.scalar.copy(vp_sb[:], vpps[:])
                    nc.vector.tensor_copy(vec3[:, c3 * HC:(c3 + 1) * HC],
                                          vp_sb[:, 2 * HC:3 * HC])
                    if c3 == 0:
                        nc.vector.tensor_tensor(vd[:], vp_sb[:, 0:HC],
                                                vp_sb[:, HC:2 * HC],
                                                op=TT.mult)
                    else:
                        vdt = fpool.tile([P, HC], F32, tag="vdt")
                        nc.vector.tensor_tensor(vdt[:], vp_sb[:, 0:HC],
                                                vp_sb[:, HC:2 * HC],
                                                op=TT.mult)
                        nc.vector.tensor_tensor(vd[:], vd[:], vdt[:],
                                                op=TT.add)

                dxs = fpool.tile([P, HC], F32, tag="dxs")
                nc.vector.tensor_tensor(dxs[:], vd[:], o_sb[:, HC:2 * HC],
                                        op=TT.mult)
                nc.vector.tensor_tensor(dxs[:], dxs[:], o_sb[:, 2 * HC:3 * HC],
                                        op=TT.add)
                nc.sync.dma_start(dx_d[s * P:(s + 1) * P, :], dxs[:])
                dvs = fpool.tile([P, HC3], F32, tag="dvs")
                for c3 in range(3):
                    sl = slice(c3 * HC, (c3 + 1) * HC)
                    nc.vector.tensor_tensor(dvs[:, sl], vec3[:, sl],
                                            o_sb[:, 0:HC], op=TT.mult)
                    nc.vector.tensor_tensor(
                        dvs[:, sl], dvs[:, sl],
                        aggs[:, HC + c3 * HC:HC + (c3 + 1) * HC], op=TT.add)
                nc.sync.dma_start(
                    dvec_d[s * P:(s + 1) * P, :, :].rearrange(
                        "n c f -> n (c f)"), dvs[:])

                off += B * P
                bb += B

        for p in (fpool, gpool, epool, npool, qpool, cpool):
            p.release()

    nc.compile()
    return nc


_CACHE = {}


def kernel(**inputs):
    from concourse.bass_utils import run_bass_kernel_spmd

    cfg, in_maps, meta = _prep(inputs, DT_MODE)
    key = (DT_MODE, tuple(cfg.B_s))
    if key not in _CACHE:
        _CACHE[key] = build(cfg)
    nc = _CACHE[key]
    res = run_bass_kernel_spmd(nc, in_maps, core_ids=list(range(CORES)),
                               trace=PROFILE, **TRACE_KW)
    n_all = np.asarray(inputs["x"]).shape[0]
    dx = np.zeros((n_all, HC), dtype=np.float32)
    dvec = np.zeros((n_all, 3, HC), dtype=np.float32)
    for c in range(CORES):
        ids = meta["node_ids"][c]
        dx[ids] = res.results[c]["dx"]
        dvec[ids] = res.results[c]["dvec"]
    kernel.last_result = res
    return dx, dvec
